# revision 2
# baseline (speedup 1.0000x reference)
"""TRN2 Bass kernel for per-sample low-rank adapter routing (moe_routing).

Computation (per batch b):
    gate  = softmax(MLP(LN(ctr[b])))              # tiny, done on host (f32)
    A     = (gate @ Wa.T).reshape(R, D_IN)        # [8, 2048]   host
    B     = (gate @ Wb.T).reshape(R, D_OUT)*scale # [8, 2048]   host
    out_b = (x_b @ A.T) @ B                       # [2048, 2048]  <- device

Device side is memory-bound: reads x (16 MiB/core) writes out (16 MiB/core).
Sharding: batch dim (8) across the 8 NeuronCores, adapters replicated.

Per-core device pipeline, per 512-row macro tile (4 per core):
  DMA x-macro [512, 2048] f32 -> SBUF (partition-major, 4 sub-tiles)
  for each of 16 contraction chunks (d):
      4x PE transpose 128x128 (f32) -> PSUM
      DVE evac PSUM -> SBUF rounding to float32r (TF32)
      f32r matmul accumulate xa^T [8, 512] (lhsT = A^T chunk, rounded)
  DVE evac xa^T -> SBUF (rounds to f32r)
  16x f32r matmul out tiles [128, 512] (rhs = B rounded)
  ACT evac PSUM -> SBUF f32; DMA out rows.

float32r matmuls run at 1 cycle/column for moving dim >= 256 (vs 4 for f32)
at ~1.5e-4 relative error (measured on HW), keeping the kernel DMA-bound.
"""
import sys

sys.path.insert(0, '/opt/trn_rl_repo')

import numpy as np

import concourse.bacc as bacc
import concourse.mybir as mybir
import concourse.tile as tile
from concourse.bass_utils import run_bass_kernel_spmd

R = 8
D_IN = 2048
D_OUT = 2048
SEQ = 2048
BS = 8
SCALING = 16.0 / R
LN_EPS = 1e-5
TEMPERATURE = 1.0

F32 = mybir.dt.float32
F32R = mybir.dt.float32r

MACRO = 512                      # seq rows per macro tile
N_MACRO = SEQ // MACRO           # 4
N_SUB = MACRO // 128             # 4 sub-tiles per macro
N_KC = D_IN // 128               # 16 contraction chunks

_COMPILED = None


def _build_program():
    nc = bacc.Bacc("TRN2", target_bir_lowering=False, debug=False, num_devices=8)
    x_d = nc.dram_tensor("x", [SEQ, D_IN], F32, kind="ExternalInput").ap()
    at_d = nc.dram_tensor("at", [D_IN, R], F32, kind="ExternalInput").ap()
    bm_d = nc.dram_tensor("bm", [R, D_OUT], F32, kind="ExternalInput").ap()
    id_d = nc.dram_tensor("ident", [128, 128], F32, kind="ExternalInput").ap()
    out_d = nc.dram_tensor("out", [SEQ, D_OUT], F32, kind="ExternalOutput").ap()

    with tile.TileContext(nc) as tc:
        with tc.tile_pool(name="const", bufs=1) as cpool, \
             tc.tile_pool(name="sb", bufs=2) as sb, \
             tc.tile_pool(name="evac", bufs=3) as evac, \
             tc.tile_pool(name="ps", bufs=2, space="PSUM") as ps:
            ident = cpool.tile([128, 128], F32, tag="ident")
            nc.sync.dma_start(ident[:], id_d[:])
            at_f = cpool.tile([128, N_KC, R], F32, tag="at_f")
            nc.sync.dma_start(at_f[:], at_d.rearrange("(c p) r -> p c r", p=128))
            at_r = cpool.tile([128, N_KC, R], F32R, tag="at_r")
            nc.vector.tensor_copy(at_r[:], at_f[:])
            bm_f = cpool.tile([R, D_OUT], F32, tag="bm_f")
            nc.sync.dma_start(bm_f[:], bm_d[:])
            bm_r = cpool.tile([R, D_OUT], F32R, tag="bm_r")
            nc.vector.tensor_copy(bm_r[:], bm_f[:])

            for m in range(N_MACRO):
                x_sb = sb.tile([128, N_SUB, D_IN], F32, tag="x_sb")
                nc.sync.dma_start(
                    x_sb[:],
                    x_d[m * MACRO:(m + 1) * MACRO, :].rearrange(
                        "(t p) d -> p t d", p=128),
                )
                xa_ps = ps.tile([R, MACRO], F32, tag="xa_ps")
                for kc in range(N_KC):
                    xt_ps = ps.tile([128, MACRO], F32, tag="xt_ps")
                    for t in range(N_SUB):
                        nc.tensor.transpose(
                            xt_ps[:, t * 128:(t + 1) * 128],
                            x_sb[:, t, kc * 128:(kc + 1) * 128],
                            ident[:],
                        )
                    xt_r = evac.tile([128, MACRO], F32R, tag="xt_r")
                    nc.vector.tensor_copy(xt_r[:], xt_ps[:])
                    nc.tensor.matmul(
                        xa_ps[:], at_r[:, kc, :], xt_r[:],
                        start=(kc == 0), stop=(kc == N_KC - 1),
                    )
                xa_r = evac.tile([R, MACRO], F32R, tag="xa_r")
                nc.vector.tensor_copy(xa_r[:], xa_ps[:])
                for t in range(N_SUB):
                    # out row block t needs all 2048 cols: 2 psum tiles of
                    # 1024 (2 banks each) so evac overlaps the next matmuls
                    o_ps = ps.tile([128, 1024], F32, tag="o_ps")
                    for j in range(2):
                        nc.tensor.matmul(
                            o_ps[:, j * 512:(j + 1) * 512],
                            xa_r[:, t * 128:(t + 1) * 128],
                            bm_r[:, j * 512:(j + 1) * 512],
                            start=True, stop=True,
                        )
                    o_sb = evac.tile([128, D_OUT], F32, tag="o_sb")
                    nc.scalar.copy(o_sb[:, 0:1024], o_ps[:])
                    o_ps2 = ps.tile([128, 1024], F32, tag="o_ps")
                    for j in range(2):
                        nc.tensor.matmul(
                            o_ps2[:, j * 512:(j + 1) * 512],
                            xa_r[:, t * 128:(t + 1) * 128],
                            bm_r[:, 1024 + j * 512:1024 + (j + 1) * 512],
                            start=True, stop=True,
                        )
                    nc.scalar.copy(o_sb[:, 1024:2048], o_ps2[:])
                    nc.sync.dma_start(
                        out_d[m * MACRO + t * 128:m * MACRO + (t + 1) * 128, :],
                        o_sb[:],
                    )
    nc.compile()
    return nc


def _gating_host(ctr, ln_gamma, ln_beta, W1, b1, W2, b2):
    """Replicates the reference gating MLP in numpy float32. ctr: [bs, 32]."""
    ctr = ctr.astype(np.float32)
    mu = np.mean(ctr, axis=-1, keepdims=True, dtype=np.float32)
    d = ctr - mu
    var = np.mean(np.square(d), axis=-1, keepdims=True, dtype=np.float32)
    z = d * (1.0 / np.sqrt(var + np.float32(LN_EPS))) * ln_gamma + ln_beta
    h = np.maximum(z @ W1.T + b1, np.float32(0.0))
    g = h @ W2.T + b2
    g = g / np.float32(TEMPERATURE)
    g = g - np.max(g, axis=-1, keepdims=True)
    e = np.exp(g)
    return (e / np.sum(e, axis=-1, keepdims=True)).astype(np.float32)


def kernel(x, ctr_hidden_states, ln_gamma, ln_beta, W1, b1, W2, b2, Wa, Wb):
    global _COMPILED
    x = np.asarray(x, dtype=np.float32)
    ctr = np.asarray(ctr_hidden_states, dtype=np.float32)
    ln_gamma = np.asarray(ln_gamma, dtype=np.float32)
    ln_beta = np.asarray(ln_beta, dtype=np.float32)
    W1 = np.asarray(W1, dtype=np.float32)
    b1 = np.asarray(b1, dtype=np.float32)
    W2 = np.asarray(W2, dtype=np.float32)
    b2 = np.asarray(b2, dtype=np.float32)
    Wa = np.asarray(Wa, dtype=np.float32)
    Wb = np.asarray(Wb, dtype=np.float32)

    gate = _gating_host(ctr, ln_gamma, ln_beta, W1, b1, W2, b2)   # [bs, 4]
    A = (gate @ Wa.T).reshape(BS, R, D_IN)                         # [bs, 8, 2048]
    Bm = (gate @ Wb.T).reshape(BS, R, D_OUT) * np.float32(SCALING)

    if _COMPILED is None:
        _COMPILED = _build_program()
    nc = _COMPILED

    ident = np.eye(128, dtype=np.float32)
    in_maps = []
    for b in range(BS):
        in_maps.append({
            "x": np.ascontiguousarray(x[b]),
            "at": np.ascontiguousarray(A[b].T),
            "bm": np.ascontiguousarray(Bm[b]),
            "ident": ident,
        })
    core_ids = list(range(BS))
    res = run_bass_kernel_spmd(nc, in_maps, core_ids)
    out = np.stack([res.results[b]["out"] for b in range(BS)], axis=0)
    return out.astype(np.float32)


# revision 3
# speedup vs baseline: 1.2718x; 1.2718x over previous
"""TRN2 Bass kernel for per-sample low-rank adapter routing (moe_routing).

Computation (per batch b):
    gate  = softmax(MLP(LN(ctr[b])))              # tiny, done on host (f32)
    A     = (gate @ Wa.T).reshape(R, D_IN)        # [8, 2048]   host
    B     = (gate @ Wb.T).reshape(R, D_OUT)*scale # [8, 2048]   host
    out_b = (x_b @ A.T) @ B                       # [2048, 2048]  <- device

Device side is memory-bound: reads x (16 MiB/core), writes out (16 MiB/core).
Sharding: batch dim (8) across the 8 NeuronCores, adapters replicated.

Layout trick: the host ships x TRANSPOSED (x^T, [d, s]) so the contraction dim
d lands on SBUF partitions straight from DMA — no on-chip transposes at all.
All matmul operands are declared float32r (TF32-like PE mode): runs at
1 cycle/column (4x faster than fp32) with ~1.5e-4 relative error, and DMA
is accepted as an f32r producer so no rounding copies are needed either.

Per-core pipeline, per 512-column macro tile (4 per core):
  DMA x^T macro [2048, 512] f32r -> SBUF [128, 16, 512]      (one dma_start)
  16x f32r matmul accumulate xa^T [8, 512] (lhsT = A^T chunk [128, 8])
  DVE evac xa^T -> SBUF f32r (rounds; tiny)
  4 row-blocks x 4 f32r matmuls out [128, 512] (rhs = B)
  PSUM -> SBUF evac split between ScalarE and VectorE; DMA out rows.
"""
import sys

sys.path.insert(0, '/opt/trn_rl_repo')

import numpy as np

import concourse.bacc as bacc
import concourse.mybir as mybir
import concourse.tile as tile
from concourse.bass_utils import run_bass_kernel_spmd

R = 8
D_IN = 2048
D_OUT = 2048
SEQ = 2048
BS = 8
SCALING = 16.0 / R
LN_EPS = 1e-5
TEMPERATURE = 1.0

F32 = mybir.dt.float32
F32R = mybir.dt.float32r

MACRO = 512                      # seq rows per macro tile
N_MACRO = SEQ // MACRO           # 4
N_SUB = MACRO // 128             # 4 row sub-blocks per macro
N_KC = D_IN // 128               # 16 contraction chunks

_COMPILED = None


def _build_program():
    nc = bacc.Bacc("TRN2", target_bir_lowering=False, debug=False, num_devices=8)
    xt_d = nc.dram_tensor("xt", [D_IN, SEQ], F32R, kind="ExternalInput").ap()
    at_d = nc.dram_tensor("at", [D_IN, R], F32R, kind="ExternalInput").ap()
    bm_d = nc.dram_tensor("bm", [R, D_OUT], F32R, kind="ExternalInput").ap()
    out_d = nc.dram_tensor("out", [SEQ, D_OUT], F32, kind="ExternalOutput").ap()

    with tile.TileContext(nc) as tc:
        with tc.tile_pool(name="const", bufs=1) as cpool, \
             tc.tile_pool(name="sb", bufs=2) as sb, \
             tc.tile_pool(name="evac", bufs=3) as evac, \
             tc.tile_pool(name="ps", bufs=2, space="PSUM") as ps:
            at_r = cpool.tile([128, N_KC, R], F32R, tag="at_r")
            nc.sync.dma_start(at_r[:], at_d.rearrange("(c p) r -> p c r", p=128))
            bm_r = cpool.tile([R, D_OUT], F32R, tag="bm_r")
            nc.sync.dma_start(bm_r[:], bm_d[:])

            for m in range(N_MACRO):
                xt_sb = sb.tile([128, N_KC, MACRO], F32R, tag="xt_sb")
                nc.sync.dma_start(
                    xt_sb[:],
                    xt_d[:, m * MACRO:(m + 1) * MACRO].rearrange(
                        "(c p) s -> p c s", p=128),
                )
                xa_ps = ps.tile([R, MACRO], F32, tag="xa_ps")
                for kc in range(N_KC):
                    nc.tensor.matmul(
                        xa_ps[:], at_r[:, kc, :], xt_sb[:, kc, :],
                        start=(kc == 0), stop=(kc == N_KC - 1),
                    )
                xa_r = evac.tile([R, MACRO], F32R, tag="xa_r")
                nc.vector.tensor_copy(xa_r[:], xa_ps[:])
                for t in range(N_SUB):
                    o_sb = evac.tile([128, D_OUT], F32, tag="o_sb")
                    for half in range(2):
                        o_ps = ps.tile([128, 1024], F32, tag="o_ps")
                        for j in range(2):
                            nc.tensor.matmul(
                                o_ps[:, j * 512:(j + 1) * 512],
                                xa_r[:, t * 128:(t + 1) * 128],
                                bm_r[:, half * 1024 + j * 512:
                                     half * 1024 + (j + 1) * 512],
                                start=True, stop=True,
                            )
                        # split psum evacuation across the two free engines
                        if half == 0:
                            nc.scalar.copy(
                                o_sb[:, half * 1024:(half + 1) * 1024], o_ps[:])
                        else:
                            nc.vector.tensor_copy(
                                o_sb[:, half * 1024:(half + 1) * 1024], o_ps[:])
                    nc.sync.dma_start(
                        out_d[m * MACRO + t * 128:m * MACRO + (t + 1) * 128, :],
                        o_sb[:],
                    )
    nc.compile()
    return nc


def _gating_host(ctr, ln_gamma, ln_beta, W1, b1, W2, b2):
    """Replicates the reference gating MLP in numpy float32. ctr: [bs, 32]."""
    ctr = ctr.astype(np.float32)
    mu = np.mean(ctr, axis=-1, keepdims=True, dtype=np.float32)
    d = ctr - mu
    var = np.mean(np.square(d), axis=-1, keepdims=True, dtype=np.float32)
    z = d * (1.0 / np.sqrt(var + np.float32(LN_EPS))) * ln_gamma + ln_beta
    h = np.maximum(z @ W1.T + b1, np.float32(0.0))
    g = h @ W2.T + b2
    g = g / np.float32(TEMPERATURE)
    g = g - np.max(g, axis=-1, keepdims=True)
    e = np.exp(g)
    return (e / np.sum(e, axis=-1, keepdims=True)).astype(np.float32)


def kernel(x, ctr_hidden_states, ln_gamma, ln_beta, W1, b1, W2, b2, Wa, Wb):
    global _COMPILED
    x = np.asarray(x, dtype=np.float32)
    ctr = np.asarray(ctr_hidden_states, dtype=np.float32)
    ln_gamma = np.asarray(ln_gamma, dtype=np.float32)
    ln_beta = np.asarray(ln_beta, dtype=np.float32)
    W1 = np.asarray(W1, dtype=np.float32)
    b1 = np.asarray(b1, dtype=np.float32)
    W2 = np.asarray(W2, dtype=np.float32)
    b2 = np.asarray(b2, dtype=np.float32)
    Wa = np.asarray(Wa, dtype=np.float32)
    Wb = np.asarray(Wb, dtype=np.float32)

    gate = _gating_host(ctr, ln_gamma, ln_beta, W1, b1, W2, b2)   # [bs, 4]
    A = (gate @ Wa.T).reshape(BS, R, D_IN)                         # [bs, 8, 2048]
    Bm = (gate @ Wb.T).reshape(BS, R, D_OUT) * np.float32(SCALING)

    if _COMPILED is None:
        _COMPILED = _build_program()
    nc = _COMPILED

    in_maps = []
    for b in range(BS):
        in_maps.append({
            "xt": np.ascontiguousarray(x[b].T),
            "at": np.ascontiguousarray(A[b].T),
            "bm": np.ascontiguousarray(Bm[b]),
        })
    core_ids = list(range(BS))
    res = run_bass_kernel_spmd(nc, in_maps, core_ids)
    out = np.stack([res.results[b]["out"] for b in range(BS)], axis=0)
    return out.astype(np.float32)


# revision 5
# speedup vs baseline: 1.2734x; 1.0013x over previous
"""TRN2 Bass kernel for per-sample low-rank adapter routing (moe_routing).

Computation (per batch b):
    gate  = softmax(MLP(LN(ctr[b])))              # tiny, done on host (f32)
    A     = (gate @ Wa.T).reshape(R, D_IN)        # [8, 2048]   host
    B     = (gate @ Wb.T).reshape(R, D_OUT)*scale # [8, 2048]   host
    out_b = (x_b @ A.T) @ B                       # [2048, 2048]  <- device

Device side is memory-bound: reads x (16 MiB/core), writes out (16 MiB/core).
Sharding: batch dim (8) across the 8 NeuronCores, adapters replicated.

Layout trick: the host ships x TRANSPOSED (x^T, [d, s]) so the contraction dim
d lands on SBUF partitions straight from DMA — no on-chip transposes at all.
All matmul operands are declared float32r (TF32-like PE mode): runs at
1 cycle/column (4x faster than fp32) with ~1.5e-4 relative error, and DMA
is accepted as an f32r producer so no rounding copies are needed either.

Per-core pipeline, per 512-column macro tile (4 per core):
  DMA x^T macro [2048, 512] f32r -> SBUF [128, 16, 512]      (one dma_start)
  16x f32r matmul accumulate xa^T [8, 512] (lhsT = A^T chunk [128, 8])
  DVE evac xa^T -> SBUF f32r (rounds; tiny)
  4 row-blocks x 4 f32r matmuls out [128, 512] (rhs = B)
  PSUM -> SBUF evac split between ScalarE and VectorE; DMA out rows.
"""
import sys

sys.path.insert(0, '/opt/trn_rl_repo')

import numpy as np

import concourse.bacc as bacc
import concourse.mybir as mybir
import concourse.tile as tile
from concourse.bass_utils import run_bass_kernel_spmd

R = 8
D_IN = 2048
D_OUT = 2048
SEQ = 2048
BS = 8
SCALING = 16.0 / R
LN_EPS = 1e-5
TEMPERATURE = 1.0

F32 = mybir.dt.float32
F32R = mybir.dt.float32r

MACRO = 512                      # seq rows per macro tile
N_MACRO = SEQ // MACRO           # 4
N_SUB = MACRO // 128             # 4 row sub-blocks per macro
N_KC = D_IN // 128               # 16 contraction chunks

_COMPILED = None


def _build_program():
    nc = bacc.Bacc("TRN2", target_bir_lowering=False, debug=False, num_devices=8)
    xt_d = nc.dram_tensor("xt", [D_IN, SEQ], F32R, kind="ExternalInput").ap()
    # host pre-permutes A^T to partition-major [128, N_KC, R]
    at_d = nc.dram_tensor("at", [128, N_KC, R], F32R, kind="ExternalInput").ap()
    bm_d = nc.dram_tensor("bm", [R, D_OUT], F32R, kind="ExternalInput").ap()
    out_d = nc.dram_tensor("out", [SEQ, D_OUT], F32, kind="ExternalOutput").ap()

    with tile.TileContext(nc) as tc:
        with tc.tile_pool(name="const", bufs=1) as cpool, \
             tc.tile_pool(name="xtp", bufs=N_KC) as xtp, \
             tc.tile_pool(name="evac", bufs=3) as evac, \
             tc.tile_pool(name="ps", bufs=2, space="PSUM") as ps:
            at_r = cpool.tile([128, N_KC, R], F32R, tag="at_r")
            nc.sync.dma_start(at_r[:], at_d[:])
            bm_r = cpool.tile([R, D_OUT], F32R, tag="bm_r")
            nc.sync.dma_start(bm_r[:], bm_d[:])

            # x^T resident in SBUF: 16 chunks [128, 2048], 8KB/partition each,
            # loaded once with fully-contiguous 8KB descriptors, reused by all
            # 4 macro passes.
            xt_sb = []
            for kc in range(N_KC):
                t_ = xtp.tile([128, SEQ], F32R, tag="xt_sb")
                nc.sync.dma_start(
                    t_[:], xt_d[kc * 128:(kc + 1) * 128, :])
                xt_sb.append(t_)

            for m in range(N_MACRO):
                xa_ps = ps.tile([R, MACRO], F32, tag="xa_ps")
                for kc in range(N_KC):
                    nc.tensor.matmul(
                        xa_ps[:],
                        at_r[:, kc, :],
                        xt_sb[kc][:, m * MACRO:(m + 1) * MACRO],
                        start=(kc == 0), stop=(kc == N_KC - 1),
                    )
                xa_r = evac.tile([R, MACRO], F32R, tag="xa_r")
                nc.vector.tensor_copy(xa_r[:], xa_ps[:])
                for t in range(N_SUB):
                    o_sb = evac.tile([128, D_OUT], F32, tag="o_sb")
                    for half in range(2):
                        o_ps = ps.tile([128, 1024], F32, tag="o_ps")
                        for j in range(2):
                            nc.tensor.matmul(
                                o_ps[:, j * 512:(j + 1) * 512],
                                xa_r[:, t * 128:(t + 1) * 128],
                                bm_r[:, half * 1024 + j * 512:
                                     half * 1024 + (j + 1) * 512],
                                start=True, stop=True,
                            )
                        # split psum evacuation across the two free engines
                        if half == 0:
                            nc.scalar.copy(
                                o_sb[:, half * 1024:(half + 1) * 1024], o_ps[:])
                        else:
                            nc.vector.tensor_copy(
                                o_sb[:, half * 1024:(half + 1) * 1024], o_ps[:])
                    nc.sync.dma_start(
                        out_d[m * MACRO + t * 128:m * MACRO + (t + 1) * 128, :],
                        o_sb[:],
                    )
    nc.compile()
    return nc


def _gating_host(ctr, ln_gamma, ln_beta, W1, b1, W2, b2):
    """Replicates the reference gating MLP in numpy float32. ctr: [bs, 32]."""
    ctr = ctr.astype(np.float32)
    mu = np.mean(ctr, axis=-1, keepdims=True, dtype=np.float32)
    d = ctr - mu
    var = np.mean(np.square(d), axis=-1, keepdims=True, dtype=np.float32)
    z = d * (1.0 / np.sqrt(var + np.float32(LN_EPS))) * ln_gamma + ln_beta
    h = np.maximum(z @ W1.T + b1, np.float32(0.0))
    g = h @ W2.T + b2
    g = g / np.float32(TEMPERATURE)
    g = g - np.max(g, axis=-1, keepdims=True)
    e = np.exp(g)
    return (e / np.sum(e, axis=-1, keepdims=True)).astype(np.float32)


def kernel(x, ctr_hidden_states, ln_gamma, ln_beta, W1, b1, W2, b2, Wa, Wb):
    global _COMPILED
    x = np.asarray(x, dtype=np.float32)
    ctr = np.asarray(ctr_hidden_states, dtype=np.float32)
    ln_gamma = np.asarray(ln_gamma, dtype=np.float32)
    ln_beta = np.asarray(ln_beta, dtype=np.float32)
    W1 = np.asarray(W1, dtype=np.float32)
    b1 = np.asarray(b1, dtype=np.float32)
    W2 = np.asarray(W2, dtype=np.float32)
    b2 = np.asarray(b2, dtype=np.float32)
    Wa = np.asarray(Wa, dtype=np.float32)
    Wb = np.asarray(Wb, dtype=np.float32)

    gate = _gating_host(ctr, ln_gamma, ln_beta, W1, b1, W2, b2)   # [bs, 4]
    A = (gate @ Wa.T).reshape(BS, R, D_IN)                         # [bs, 8, 2048]
    Bm = (gate @ Wb.T).reshape(BS, R, D_OUT) * np.float32(SCALING)

    if _COMPILED is None:
        _COMPILED = _build_program()
    nc = _COMPILED

    in_maps = []
    for b in range(BS):
        # at: A^T [2048, 8] -> partition-major [128, N_KC, R]
        at_pm = np.ascontiguousarray(
            A[b].T.reshape(N_KC, 128, R).transpose(1, 0, 2))
        in_maps.append({
            "xt": np.ascontiguousarray(x[b].T),
            "at": at_pm,
            "bm": np.ascontiguousarray(Bm[b]),
        })
    core_ids = list(range(BS))
    res = run_bass_kernel_spmd(nc, in_maps, core_ids)
    out = np.stack([res.results[b]["out"] for b in range(BS)], axis=0)
    return out.astype(np.float32)
